# revision 1
# baseline (speedup 1.0000x reference)
"""Trainium2 Bass kernel for nn_CriticHead (critic head over C*t tasks).

Contract: kernel(**inputs) takes the FULL unsharded inputs (as produced by
setup_inputs()) and returns the FULL [1, T] float32 output.  Internally the
work is sharded data-parallel over the leading cluster axis across 8
NeuronCores; the tiny MLP weights are replicated.

Math (per task j, verified against the reference):
    me_j   = mean(enode[j,:])                       # since y41 = y2 * me
    sc_j   = sum(ccl[j,:]) * sum(cnd[j,:])          # since y42 = y2 * sc
    u_j    = [outer3(res_j, fr_j, estep_j) (150) ; bb_j (768)]   # 918
    y2_j   = relu(W1.T u_j + b1)                    # 128
    a3     = me*(y2@W3)+b3 ; a5 = sc*(y2@W5)+b5     # sigmoid-gated pair
    a4     = me*(y2@W4)+b4 ; a6 = sc*(y2@W6)+b6     # linear pair
    p      = sig(a3)*sig(a5)
    y      = FAILC + p*((a4+a6) - FAILC)
"""

import sys

if "/opt/trn_rl_repo" not in sys.path:
    sys.path.insert(0, "/opt/trn_rl_repo")

from contextlib import ExitStack

import numpy as np

import concourse.bass as bass
import concourse.mybir as mybir
import concourse.tile as tile
from concourse.bass_utils import run_bass_kernel_spmd

# Problem constants (hardcoded per the harness contract).
NCORES = 8
C, TASKS = 64, 64
T = C * TASKS                 # 4096
TC = T // NCORES              # 512 tasks per core
D_BB = 768
N_OUT = 150                   # 5*5*6 outer-product features
D_IN = N_OUT + D_BB           # 918
D_H = 128
E_N = 64                      # edge nodes
C_C, C_N = 4, 32              # cloud clusters / nodes
N_AGG = E_N + C_C + C_N       # 100
FAILC = -100.0
NTILE = TC // 128             # 4 task tiles of 128 per core

F32 = mybir.dt.float32
BF16 = mybir.dt.bfloat16
F32R = mybir.dt.float32r

# K-tiling of the 918-row contraction: rows 0:150 are outer3, 150:918 are bb.
KT_ROWS = [128, 128, 128, 128, 128, 128, 128, 22]
KT_STARTS = [0, 128, 256, 384, 512, 640, 768, 896]


# rfeT row layout: 0:6 estep, 6:11 res, 11:16 fr (estep must start at
# partition 0 — compute-engine operands need 32-aligned base partitions).
RFE_ESTEP, RFE_RES, RFE_FR = 0, 6, 11


def _build_module(mm_dtype=BF16):
    nc = bass.Bass()

    bbh = nc.declare_dram_parameter("bbh", [D_BB, TC], BF16, isOutput=False)
    bbl = nc.declare_dram_parameter("bbl", [D_BB, TC], BF16, isOutput=False)
    rfeT = nc.declare_dram_parameter("rfeT", [16, TC], BF16, isOutput=False)
    aggT = nc.declare_dram_parameter("aggT", [N_AGG, TC], F32, isOutput=False)
    w1bh = nc.declare_dram_parameter("w1bh", [D_BB, D_H], BF16, isOutput=False)
    w1bl = nc.declare_dram_parameter("w1bl", [D_BB, D_H], BF16, isOutput=False)
    w1a = nc.declare_dram_parameter("w1a", [N_OUT, D_H], BF16, isOutput=False)
    wh = nc.declare_dram_parameter("wh", [D_H, 4], F32, isOutput=False)
    wa = nc.declare_dram_parameter("wa", [N_AGG, 3], F32, isOutput=False)
    brep = nc.declare_dram_parameter("brep", [16, 180], BF16, isOutput=False)
    b1 = nc.declare_dram_parameter("b1", [D_H, 1], F32, isOutput=False)
    bh4 = nc.declare_dram_parameter("bh4", [1, 4], F32, isOutput=False)
    out = nc.declare_dram_parameter("out", [128, NTILE], F32, isOutput=True)

    with tile.TileContext(nc) as tc, ExitStack() as ctx:
        singles = ctx.enter_context(tc.tile_pool(name="singles", bufs=1))
        work = ctx.enter_context(tc.tile_pool(name="work", bufs=1))
        small = ctx.enter_context(tc.tile_pool(name="small", bufs=1))
        psum = ctx.enter_context(tc.tile_pool(name="psum", bufs=1, space="PSUM"))


        # Preload the sigmoid ACT table early (overlaps the big DMAs) so the
        # real sigmoid near the kernel tail doesn't pay the 1.3us table load.
        sgw = small.tile([32, 1], F32, tag="sgw")
        nc.vector.memset(sgw, 0.0)
        nc.scalar.activation(sgw, sgw, mybir.ActivationFunctionType.Sigmoid)

        # ---- chunked big loads (one DMA each, issued from SP/HWDGE) ------
        bbh_s = work.tile([128, 6, TC], BF16, tag="bbh")
        nc.sync.dma_start(out=bbh_s, in_=bbh[:, :].rearrange("(j p) t -> p j t", p=128))
        bbl_s = work.tile([128, 6, TC], BF16, tag="bbl")
        nc.sync.dma_start(out=bbl_s, in_=bbl[:, :].rearrange("(j p) t -> p j t", p=128))
        w1bh_s = singles.tile([128, 6, D_H], BF16, tag="w1bh")
        nc.sync.dma_start(
            out=w1bh_s, in_=w1bh[:, :].rearrange("(j p) h -> p j h", p=128)
        )
        w1bl_s = singles.tile([128, 6, D_H], BF16, tag="w1bl")
        nc.sync.dma_start(
            out=w1bl_s, in_=w1bl[:, :].rearrange("(j p) h -> p j h", p=128)
        )
        agg_s = singles.tile([N_AGG, TC], F32, tag="agg")
        nc.sync.dma_start(out=agg_s, in_=aggT[:, :])

        # ---- small loads spread across other engine queues ---------------
        rfe_s = singles.tile([16, TC], BF16, tag="rfe")
        nc.sync.dma_start(out=rfe_s, in_=rfeT[:, :])
        brep_s = singles.tile([16, 180], BF16, tag="brep")
        nc.sync.dma_start(out=brep_s, in_=brep[:, :])
        bh_s = singles.tile([128, 4], F32, tag="bh")
        nc.gpsimd.dma_start(out=bh_s, in_=bh4[:, :].partition_broadcast(128))
        w1a0 = singles.tile([128, D_H], BF16, tag="w1a0")
        nc.scalar.dma_start(out=w1a0, in_=w1a[0:128, :])
        w1a1 = singles.tile([22, D_H], BF16, tag="w1a1")
        nc.scalar.dma_start(out=w1a1, in_=w1a[128:150, :])
        wh_s = singles.tile([D_H, 4], F32, tag="wh")
        nc.scalar.dma_start(out=wh_s, in_=wh[:, :])
        wa_s = singles.tile([N_AGG, 3], F32, tag="wa")
        nc.scalar.dma_start(out=wa_s, in_=wa[:, :])
        b1_s = singles.tile([D_H, 1], F32, tag="b1")
        nc.scalar.dma_start(out=b1_s, in_=b1[:, :])

        # ---- outer3 features: u rows 0:150 as kt0 [128] + kt1 [22] -------
        ps_fr = psum.tile([30, TC], F32, tag="ps_fr")
        nc.tensor.matmul(ps_fr, lhsT=brep_s[:, 0:30], rhs=rfe_s, start=True, stop=True)
        ps_r0 = psum.tile([128, TC], F32, tag="ps_r0")
        nc.tensor.matmul(
            ps_r0, lhsT=brep_s[:, 30:158], rhs=rfe_s, start=True, stop=True
        )
        ps_r1 = psum.tile([22, TC], F32, tag="ps_r1")
        nc.tensor.matmul(
            ps_r1, lhsT=brep_s[:, 158:180], rhs=rfe_s, start=True, stop=True
        )

        estp = small.tile([6, TC], F32, tag="estp")
        nc.vector.tensor_copy(estp, rfe_s[RFE_ESTEP : RFE_ESTEP + 6, :])
        estpT = small.tile([30, TC], F32, tag="estpT")
        for m in range(5):
            nc.sync.dma_start(out=estpT[6 * m : 6 * m + 6, :], in_=estp)

        fe = small.tile([30, TC], F32, tag="fe")
        nc.vector.tensor_mul(fe, ps_fr, estpT)

        feT0 = work.tile([128, TC], F32, tag="feT0")
        for q in range(4):
            eng = nc.sync if q % 2 == 0 else nc.scalar
            eng.dma_start(out=feT0[30 * q : 30 * q + 30, :], in_=fe)
        nc.scalar.dma_start(out=feT0[120:128, :], in_=fe[0:8, :])
        feT1 = work.tile([22, TC], F32, tag="feT1")
        nc.scalar.dma_start(out=feT1, in_=fe[8:30, :])

        kt0 = work.tile([128, TC], BF16, tag="kt0")
        nc.vector.tensor_mul(kt0, feT0, ps_r0)
        kt1 = work.tile([22, TC], BF16, tag="kt1")
        nc.vector.tensor_mul(kt1, feT1, ps_r1)

        # ---- main matmul: y2T += W1h.T uh + W1l.T uh + W1h.T ul ----------
        psumY = psum.tile([128, TC], F32, tag="psumY")
        n_mm = 20
        pos = 0
        for j in range(6):
            nc.tensor.matmul(
                psumY, lhsT=w1bh_s[:, j, :], rhs=bbh_s[:, j, :],
                start=(pos == 0), stop=(pos == n_mm - 1))
            pos += 1
        for j in range(6):
            nc.tensor.matmul(
                psumY, lhsT=w1bl_s[:, j, :], rhs=bbh_s[:, j, :],
                start=(pos == 0), stop=(pos == n_mm - 1))
            pos += 1
        for j in range(6):
            nc.tensor.matmul(
                psumY, lhsT=w1bh_s[:, j, :], rhs=bbl_s[:, j, :],
                start=(pos == 0), stop=(pos == n_mm - 1))
            pos += 1
        nc.tensor.matmul(psumY, lhsT=w1a0, rhs=kt0,
                         start=(pos == 0), stop=(pos == n_mm - 1))
        pos += 1
        nc.tensor.matmul(psumY, lhsT=w1a1, rhs=kt1,
                         start=(pos == 0), stop=(pos == n_mm - 1))

        y2T = work.tile([128, TC], F32, tag="y2T")
        nc.scalar.activation(
            y2T, psumY, mybir.ActivationFunctionType.Relu, bias=b1_s, scale=1.0
        )

        # ---- heads, task-major: one 128-task tile at a time --------------
        psumS = psum.tile([128, NTILE, 7], F32, tag="psumS")
        for i in range(NTILE):
            nc.tensor.matmul(
                psumS[:, i, 0:4],
                lhsT=y2T[:, 128 * i : 128 * (i + 1)],
                rhs=wh_s,
                start=True,
                stop=True,
            )
            nc.tensor.matmul(
                psumS[:, i, 4:7],
                lhsT=agg_s[:, 128 * i : 128 * (i + 1)],
                rhs=wa_s,
                start=True,
                stop=True,
            )

        # ---- combine ------------------------------------------------------
        # cols of psumS[:, i, :]: d3, d5, d4, d6, me, sum_ccl, sum_cnd
        mes = small.tile([128, NTILE, 3], F32, tag="mes")
        nc.vector.tensor_copy(mes, psumS[:, :, 4:7])
        g2 = small.tile([128, NTILE, 2], F32, tag="g2")
        nc.vector.tensor_copy(g2[:, :, 0:1], mes[:, :, 0:1])
        nc.vector.tensor_mul(g2[:, :, 1:2], mes[:, :, 1:2], mes[:, :, 2:3])

        av = small.tile([128, NTILE, 4], F32, tag="av")
        nc.vector.tensor_mul(av[:, :, 0:2], psumS[:, :, 0:2], g2)
        nc.vector.tensor_mul(av[:, :, 2:4], psumS[:, :, 2:4], g2)
        nc.vector.tensor_add(
            av, av, bh_s.unsqueeze(1).broadcast_to([128, NTILE, 4])
        )

        sg = small.tile([128, NTILE, 2], F32, tag="sg")
        nc.scalar.activation(sg, av[:, :, 0:2], mybir.ActivationFunctionType.Sigmoid)

        y6s = small.tile([128, NTILE, 1], F32, tag="y6s")
        nc.vector.tensor_add(y6s, av[:, :, 2:3], av[:, :, 3:4])
        pv = small.tile([128, NTILE, 1], F32, tag="pv")
        nc.vector.tensor_mul(pv, sg[:, :, 0:1], sg[:, :, 1:2])
        tt = small.tile([128, NTILE, 1], F32, tag="tt")
        nc.vector.scalar_tensor_tensor(
            out=tt,
            in0=y6s,
            scalar=FAILC,
            in1=pv,
            op0=mybir.AluOpType.subtract,
            op1=mybir.AluOpType.mult,
        )
        outv = small.tile([128, NTILE, 1], F32, tag="outv")
        nc.vector.tensor_scalar_add(outv, tt, FAILC)

        nc.sync.dma_start(out=out[:, :], in_=outv[:, :, 0])

    return _split_sync_waits(nc)


def _split_sync_waits(nc, max_waits=1):
    """This container's walrus rejects >1 sem-wait per instruction
    ("Too many sync wait commands"); hoist extras onto same-engine NOPs."""
    nid = 0
    for f in nc.m.functions:
        for bb in f.blocks:
            new = []
            for inst in bb.instructions:
                si = inst.sync_info
                if si is None:
                    new.append(inst)
                    continue
                waits = list(si.on_wait or [])
                if len(waits) > max_waits:
                    for w in waits[:-max_waits]:
                        nop = mybir.InstNoOp(name=f"WSPL-{nid}", ins=[], outs=[])
                        nid += 1
                        nop.engine = inst.engine
                        nop.sync_info = mybir.SyncInfo(on_wait=[w], on_update=[])
                        new.append(nop)
                    inst.sync_info = mybir.SyncInfo(
                        on_wait=waits[-max_waits:], on_update=list(si.on_update or [])
                    )
                new.append(inst)
            bb.instructions = new
    return nc


_CACHED_NC = None


def _get_nc():
    global _CACHED_NC
    if _CACHED_NC is None:
        _CACHED_NC = _build_module()
    return _CACHED_NC


def _make_in_maps(inputs: dict) -> list[dict[str, np.ndarray]]:
    f32 = np.float32
    bf16 = np.dtype("bfloat16")

    bb = np.asarray(inputs["backbone_y"], f32).reshape(T, D_BB)
    res = np.asarray(inputs["y_res"], f32).reshape(T, 5)
    fr = np.asarray(inputs["y_fr"], f32).reshape(T, 5)
    estep = np.asarray(inputs["y_estep"], f32).reshape(T, 6)
    enode = np.asarray(inputs["y_enode"], f32).reshape(T, E_N)
    ccl = np.asarray(inputs["y_ccluster"], f32).reshape(T, C_C)
    cnd = np.asarray(inputs["y_cnode"], f32).reshape(T, C_N)

    w1 = np.ascontiguousarray(np.asarray(inputs["W1"], f32))
    w1a = np.ascontiguousarray(w1[0:N_OUT].astype(bf16))
    w1b = w1[N_OUT:]
    w1bh = np.ascontiguousarray(w1b.astype(bf16))
    w1bl = np.ascontiguousarray((w1b - w1bh.astype(f32)).astype(bf16))
    b1 = np.ascontiguousarray(np.asarray(inputs["b1"], f32).reshape(D_H, 1))
    w3 = np.asarray(inputs["W3"], f32).reshape(D_H, 1)
    w4 = np.asarray(inputs["W4"], f32).reshape(D_H, 1)
    w5 = np.asarray(inputs["W5"], f32).reshape(D_H, 1)
    w6 = np.asarray(inputs["W6"], f32).reshape(D_H, 1)
    # col order: d3, d5 (sigmoid-gated), d4, d6 (linear)
    wh = np.ascontiguousarray(np.concatenate([w3, w5, w4, w6], axis=1))
    bh = np.array(
        [
            [
                float(np.asarray(inputs["b3"]).reshape(-1)[0]),
                float(np.asarray(inputs["b5"]).reshape(-1)[0]),
                float(np.asarray(inputs["b4"]).reshape(-1)[0]),
                float(np.asarray(inputs["b6"]).reshape(-1)[0]),
            ]
        ],
        f32,
    )

    wa = np.zeros((N_AGG, 3), f32)
    wa[0:E_N, 0] = 1.0 / E_N
    wa[E_N : E_N + C_C, 1] = 1.0
    wa[E_N + C_C :, 2] = 1.0

    brep = np.zeros((16, 180), f32)
    for m in range(5):
        for o in range(6):
            brep[RFE_FR + m, m * 6 + o] = 1.0
    for r in range(128):
        brep[RFE_RES + r // 30, 30 + r] = 1.0
    for j in range(22):
        brep[RFE_RES + 4, 158 + j] = 1.0
    brep = brep.astype(bf16)

    rfe = np.concatenate([estep, res, fr], axis=1)  # [T, 16]

    in_maps = []
    for c in range(NCORES):
        sl = slice(c * TC, (c + 1) * TC)
        bbT_c = bb[sl].T  # [768, TC] f32
        bbh_c = bbT_c.astype(bf16)
        bbl_c = (bbT_c - bbh_c.astype(f32)).astype(bf16)
        in_maps.append(
            {
                "bbh": np.ascontiguousarray(bbh_c),
                "bbl": np.ascontiguousarray(bbl_c),
                "rfeT": np.ascontiguousarray(rfe[sl].T.astype(bf16)),
                "aggT": np.ascontiguousarray(
                    np.concatenate([enode[sl], ccl[sl], cnd[sl]], axis=1).T
                ),
                "w1bh": w1bh,
                "w1bl": w1bl,
                "w1a": w1a,
                "wh": wh,
                "wa": wa,
                "brep": brep,
                "b1": b1,
                "bh4": bh,
            }
        )
    return in_maps


def _assemble(results: list[dict[str, np.ndarray]]) -> np.ndarray:
    parts = [np.asarray(results[c]["out"]).T.reshape(-1) for c in range(NCORES)]
    return np.concatenate(parts)[None, :].astype(np.float32)


def _run(inputs: dict, trace: bool = False):
    nc = _get_nc()
    in_maps = _make_in_maps(inputs)
    kres = run_bass_kernel_spmd(
        nc, in_maps, core_ids=list(range(NCORES)), trace=trace
    )
    return _assemble(kres.results), kres


def kernel(**inputs) -> np.ndarray:
    out, _ = _run(inputs)
    return out



# revision 6
# speedup vs baseline: 1.6428x; 1.6428x over previous
"""Trainium2 Bass kernel for nn_CriticHead (critic head over C*t tasks).

Contract: kernel(**inputs) takes the FULL unsharded inputs (as produced by
setup_inputs()) and returns the FULL [1, T] float32 output.  Internally the
work is sharded data-parallel over the leading cluster axis across 8
NeuronCores; the tiny MLP weights are replicated.

Math (per task j, verified against the reference):
    me_j   = mean(enode[j,:])                       # since y41 = y2 * me
    sc_j   = sum(ccl[j,:]) * sum(cnd[j,:])          # since y42 = y2 * sc
    u_j    = [outer3(res_j, fr_j, estep_j) (150) ; bb_j (768)]   # 918
    y2_j   = relu(W1.T u_j + b1)                    # 128
    a3     = me*(y2@W3)+b3 ; a5 = sc*(y2@W5)+b5     # sigmoid-gated pair
    a4     = me*(y2@W4)+b4 ; a6 = sc*(y2@W6)+b6     # linear pair
    p      = sig(a3)*sig(a5)
    y      = FAILC + p*((a4+a6) - FAILC)

v2 design (all-fp16 data path, ~0.4% rel err):
  - Everything streams in fp16: halves backbone DMA vs fp32/bf16-split and
    every matmul runs at 1 cycle/row on the PE.
  - The 150 outer3 features are built on the PE: three selection matmuls
    replicate res/fr/estep rows of rfe into [150, TC] layouts (two chunks,
    128+22), then two vector muls form the products.  No sbuf->sbuf
    broadcast DMAs (each dma_start costs ~0.7us of engine issue time).
  - bb streams in 3 chunks on its own queue so the 6 main matmul passes
    pipeline with the load.
  - Head matmuls run in fp16 task-major (cheap LDWEIGHTS), biases are baked
    at build time (emitted only when nonzero).
"""

import sys

if "/opt/trn_rl_repo" not in sys.path:
    sys.path.insert(0, "/opt/trn_rl_repo")

from contextlib import ExitStack

import numpy as np

import concourse.bass as bass
import concourse.mybir as mybir
import concourse.tile as tile
from concourse.bass_utils import run_bass_kernel_spmd

# Problem constants (hardcoded per the harness contract).
NCORES = 8
C, TASKS = 64, 64
T = C * TASKS                 # 4096
TC = T // NCORES              # 512 tasks per core
D_BB = 768
N_OUT = 150                   # 5*5*6 outer-product features
D_H = 128
E_N = 64                      # edge nodes
C_C, C_N = 4, 32              # cloud clusters / nodes
N_AGG = E_N + C_C + C_N       # 100
FAILC = -100.0
NTILE = TC // 128             # 4 task tiles of 128 per core
NBBC = 3                      # bb streamed in 3 chunks of [128, 2, TC]

F32 = mybir.dt.float32
F16 = mybir.dt.float16

# rfe row layout: 0:6 estep, 6:11 res, 11:16 fr.
RFE_ESTEP, RFE_RES, RFE_FR = 0, 6, 11


def _build_module(b1_vec, b3, b4, b5, b6):
    has_b1 = bool(np.any(b1_vec != 0.0))
    nc = bass.Bass()

    bbT = nc.declare_dram_parameter("bbT", [D_BB, TC], F16, isOutput=False)
    w1b = nc.declare_dram_parameter("w1b", [D_BB, D_H], F16, isOutput=False)
    w1a = nc.declare_dram_parameter("w1a", [256, D_H], F16, isOutput=False)
    rfeT = nc.declare_dram_parameter("rfeT", [16, TC], F16, isOutput=False)
    sel = nc.declare_dram_parameter("sel", [16, 3 * N_OUT], F16, isOutput=False)
    aggT = nc.declare_dram_parameter("aggT", [N_AGG, TC], F16, isOutput=False)
    whwa = nc.declare_dram_parameter("whwa", [D_H, 7], F16, isOutput=False)
    if has_b1:
        b1p = nc.declare_dram_parameter("b1", [D_H, 1], F32, isOutput=False)
    out = nc.declare_dram_parameter("out", [128, NTILE], F32, isOutput=True)

    with tile.TileContext(nc) as tc, ExitStack() as ctx:
        singles = ctx.enter_context(tc.tile_pool(name="singles", bufs=1))
        work = ctx.enter_context(tc.tile_pool(name="work", bufs=1))
        small = ctx.enter_context(tc.tile_pool(name="small", bufs=1))
        psum = ctx.enter_context(tc.tile_pool(name="psum", bufs=1, space="PSUM"))

        # ---- DMAs: small/critical tensors first, spread across queues ----
        # scalar queue: the outer3-feature inputs (needed first by the PE).
        rfe_s = singles.tile([16, TC], F16, tag="rfe")
        nc.scalar.dma_start(out=rfe_s, in_=rfeT[:, :])
        sel_s = singles.tile([16, 3 * N_OUT], F16, tag="sel")
        nc.scalar.dma_start(out=sel_s, in_=sel[:, :])

        # sync queue: main-matmul weights then the 3 bb chunks.
        w1b_s = singles.tile([128, 6, D_H], F16, tag="w1b")
        nc.sync.dma_start(
            out=w1b_s, in_=w1b[:, :].rearrange("(j p) h -> p j h", p=128)
        )
        bbc = []
        for cb in range(NBBC):
            t_ = work.tile([128, 2, TC], F16, tag=f"bbc{cb}")
            nc.sync.dma_start(
                out=t_,
                in_=bbT[256 * cb : 256 * (cb + 1), :].rearrange(
                    "(j p) t -> p j t", p=128
                ),
            )
            bbc.append(t_)

        # gpsimd queue: tail-stage tensors.
        agg_s = singles.tile([N_AGG, TC], F16, tag="agg")
        nc.gpsimd.dma_start(out=agg_s, in_=aggT[:, :])
        whwa_s = singles.tile([D_H, 7], F16, tag="whwa")
        nc.gpsimd.dma_start(out=whwa_s, in_=whwa[:, :])

        # scalar queue (cont.): kt-contraction weights, then the ACT table
        # preload (overlaps the DMAs) so the real sigmoid near the kernel
        # tail doesn't pay the 1.3us table load.
        w1a_s = singles.tile([128, 2, D_H], F16, tag="w1a")
        nc.scalar.dma_start(
            out=w1a_s, in_=w1a[:, :].rearrange("(j p) h -> p j h", p=128)
        )
        if has_b1:
            b1_s = singles.tile([D_H, 1], F32, tag="b1")
            nc.scalar.dma_start(out=b1_s, in_=b1p[:, :])
        sgw = small.tile([32, 1], F32, tag="sgw")
        nc.vector.memset(sgw, 0.0)
        nc.scalar.activation(sgw, sgw, mybir.ActivationFunctionType.Sigmoid)

        # ---- outer3 features on the PE: replicate res/fr/estep rows -------
        # kt row r = n*30 + m*6 + o  ->  res_n * fr_m * estep_o
        S_RES, S_FR, S_ES = 0, N_OUT, 2 * N_OUT
        psA = psum.tile([128, TC], F32, tag="psA")  # res, rows 0:128
        psB = psum.tile([128, TC], F32, tag="psB")  # fr,  rows 0:128
        psC = psum.tile([128, TC], F32, tag="psC")  # estep, rows 0:128
        psD = psum.tile([128, TC], F32, tag="psD")  # rows 128:150 of all 3
        nc.tensor.matmul(
            psA, lhsT=sel_s[:, S_RES : S_RES + 128], rhs=rfe_s, start=True, stop=True
        )
        nc.tensor.matmul(
            psB, lhsT=sel_s[:, S_FR : S_FR + 128], rhs=rfe_s, start=True, stop=True
        )
        nc.tensor.matmul(
            psC, lhsT=sel_s[:, S_ES : S_ES + 128], rhs=rfe_s, start=True, stop=True
        )
        nc.tensor.matmul(
            psD[0:22, :], lhsT=sel_s[:, S_RES + 128 : S_RES + 150], rhs=rfe_s,
            start=True, stop=True,
        )
        nc.tensor.matmul(
            psD[32:54, :], lhsT=sel_s[:, S_FR + 128 : S_FR + 150], rhs=rfe_s,
            start=True, stop=True,
        )
        nc.tensor.matmul(
            psD[64:86, :], lhsT=sel_s[:, S_ES + 128 : S_ES + 150], rhs=rfe_s,
            start=True, stop=True,
        )

        # vector/scalar ops may read at most ONE psum operand: stage the
        # res replica through sbuf on the (otherwise idle) scalar engine.
        a0 = work.tile([128, TC], F32, tag="a0")
        nc.scalar.activation(a0, psA, mybir.ActivationFunctionType.Copy)
        a1 = small.tile([22, TC], F32, tag="a1")
        nc.scalar.activation(a1, psD[0:22, :], mybir.ActivationFunctionType.Copy)

        t0 = work.tile([128, TC], F32, tag="t0")
        nc.vector.tensor_mul(t0, a0, psB)
        kt0 = work.tile([128, TC], F16, tag="kt0")
        nc.vector.tensor_mul(kt0, t0, psC)
        t1 = small.tile([22, TC], F32, tag="t1")
        nc.vector.tensor_mul(t1, a1, psD[32:54, :])
        kt1 = small.tile([22, TC], F16, tag="kt1")
        nc.vector.tensor_mul(kt1, t1, psD[64:86, :])

        # ---- main matmul: y2T = relu(W1.T u + b1), u = [kt ; bb] ----------
        psumY = psum.tile([128, TC], F32, tag="psumY")
        for cb in range(NBBC):
            for j in range(2):
                nc.tensor.matmul(
                    psumY, lhsT=w1b_s[:, 2 * cb + j, :], rhs=bbc[cb][:, j, :],
                    start=(cb == 0 and j == 0), stop=False,
                )
        nc.tensor.matmul(psumY, lhsT=w1a_s[:, 0, :], rhs=kt0, start=False, stop=False)
        nc.tensor.matmul(
            psumY, lhsT=w1a_s[0:22, 1, :], rhs=kt1, start=False, stop=True
        )

        y2 = work.tile([128, TC], F16, tag="y2")
        if has_b1:
            nc.scalar.activation(
                y2, psumY, mybir.ActivationFunctionType.Relu, bias=b1_s, scale=1.0
            )
        else:
            nc.scalar.activation(y2, psumY, mybir.ActivationFunctionType.Relu)

        # ---- heads, task-major: one 128-task tile at a time ---------------
        # cols of psumS[:, i, :]: d3, d5, d4, d6, me, sum_ccl, sum_cnd
        psumS = psum.tile([128, NTILE, 7], F32, tag="psumS")
        for i in range(NTILE):
            nc.tensor.matmul(
                psumS[:, i, 0:4],
                lhsT=y2[:, 128 * i : 128 * (i + 1)],
                rhs=whwa_s[:, 0:4],
                start=True,
                stop=True,
            )
            nc.tensor.matmul(
                psumS[:, i, 4:7],
                lhsT=agg_s[:, 128 * i : 128 * (i + 1)],
                rhs=whwa_s[0:N_AGG, 4:7],
                start=True,
                stop=True,
            )

        # ---- combine ------------------------------------------------------
        mes = small.tile([128, NTILE, 3], F32, tag="mes")
        nc.vector.tensor_copy(mes, psumS[:, :, 4:7])
        g2 = small.tile([128, NTILE, 2], F32, tag="g2")
        nc.vector.tensor_copy(g2[:, :, 0:1], mes[:, :, 0:1])
        nc.vector.tensor_mul(g2[:, :, 1:2], mes[:, :, 1:2], mes[:, :, 2:3])

        av = small.tile([128, NTILE, 4], F32, tag="av")
        nc.vector.tensor_mul(av[:, :, 0:2], psumS[:, :, 0:2], g2)
        nc.vector.tensor_mul(av[:, :, 2:4], psumS[:, :, 2:4], g2)
        if b3 != 0.0:
            nc.vector.tensor_scalar_add(av[:, :, 0:1], av[:, :, 0:1], float(b3))
        if b5 != 0.0:
            nc.vector.tensor_scalar_add(av[:, :, 1:2], av[:, :, 1:2], float(b5))

        sg = small.tile([128, NTILE, 2], F32, tag="sg")
        nc.scalar.activation(sg, av[:, :, 0:2], mybir.ActivationFunctionType.Sigmoid)

        y6s = small.tile([128, NTILE, 1], F32, tag="y6s")
        nc.vector.tensor_add(y6s, av[:, :, 2:3], av[:, :, 3:4])
        if (b4 + b6) != 0.0:
            nc.vector.tensor_scalar_add(y6s, y6s, float(b4 + b6))
        pv = small.tile([128, NTILE, 1], F32, tag="pv")
        nc.vector.tensor_mul(pv, sg[:, :, 0:1], sg[:, :, 1:2])
        tt = small.tile([128, NTILE, 1], F32, tag="tt")
        nc.vector.scalar_tensor_tensor(
            out=tt,
            in0=y6s,
            scalar=FAILC,
            in1=pv,
            op0=mybir.AluOpType.subtract,
            op1=mybir.AluOpType.mult,
        )
        outv = small.tile([128, NTILE, 1], F32, tag="outv")
        nc.vector.tensor_scalar_add(outv, tt, FAILC)

        nc.gpsimd.dma_start(out=out[:, :], in_=outv[:, :, 0])

    return _split_sync_waits(nc)


def _split_sync_waits(nc, max_waits=1):
    """This container's walrus rejects >1 sem-wait per instruction
    ("Too many sync wait commands"); hoist extras onto same-engine NOPs."""
    nid = 0
    for f in nc.m.functions:
        for bb in f.blocks:
            new = []
            for inst in bb.instructions:
                si = inst.sync_info
                if si is None:
                    new.append(inst)
                    continue
                waits = list(si.on_wait or [])
                if len(waits) > max_waits:
                    for w in waits[:-max_waits]:
                        nop = mybir.InstNoOp(name=f"WSPL-{nid}", ins=[], outs=[])
                        nid += 1
                        nop.engine = inst.engine
                        nop.sync_info = mybir.SyncInfo(on_wait=[w], on_update=[])
                        new.append(nop)
                    inst.sync_info = mybir.SyncInfo(
                        on_wait=waits[-max_waits:], on_update=list(si.on_update or [])
                    )
                new.append(inst)
            bb.instructions = new
    return nc


_CACHED_NC = {}


def _get_nc(b1_vec, b3, b4, b5, b6):
    key = (bool(np.any(b1_vec != 0.0)), float(b3), float(b4), float(b5), float(b6))
    if key not in _CACHED_NC:
        _CACHED_NC[key] = _build_module(b1_vec, b3, b4, b5, b6)
    return _CACHED_NC[key]


def _make_in_maps(inputs: dict) -> list[dict[str, np.ndarray]]:
    f32 = np.float32
    f16 = np.float16

    bb = np.asarray(inputs["backbone_y"], f32).reshape(T, D_BB)
    res = np.asarray(inputs["y_res"], f32).reshape(T, 5)
    fr = np.asarray(inputs["y_fr"], f32).reshape(T, 5)
    estep = np.asarray(inputs["y_estep"], f32).reshape(T, 6)
    enode = np.asarray(inputs["y_enode"], f32).reshape(T, E_N)
    ccl = np.asarray(inputs["y_ccluster"], f32).reshape(T, C_C)
    cnd = np.asarray(inputs["y_cnode"], f32).reshape(T, C_N)

    w1 = np.ascontiguousarray(np.asarray(inputs["W1"], f32))
    w1b_h = np.ascontiguousarray(w1[N_OUT:].astype(f16))
    w1a_h = np.zeros((256, D_H), f16)
    w1a_h[0:N_OUT] = w1[0:N_OUT].astype(f16)
    w3 = np.asarray(inputs["W3"], f32).reshape(D_H, 1)
    w4 = np.asarray(inputs["W4"], f32).reshape(D_H, 1)
    w5 = np.asarray(inputs["W5"], f32).reshape(D_H, 1)
    w6 = np.asarray(inputs["W6"], f32).reshape(D_H, 1)
    # head col order: d3, d5 (sigmoid-gated), d4, d6 (linear); cols 4:7 agg.
    whwa_h = np.zeros((D_H, 7), f16)
    whwa_h[:, 0:4] = np.concatenate([w3, w5, w4, w6], axis=1).astype(f16)
    whwa_h[0:E_N, 4] = np.float16(1.0 / E_N)
    whwa_h[E_N : E_N + C_C, 5] = np.float16(1.0)
    whwa_h[E_N + C_C : N_AGG, 6] = np.float16(1.0)

    # selection matrices: kt row r = n*30+m*6+o -> res_n * fr_m * estep_o
    sel_h = np.zeros((16, 3 * N_OUT), f16)
    for r in range(N_OUT):
        n, mo = divmod(r, 30)
        m, o = divmod(mo, 6)
        sel_h[RFE_RES + n, r] = 1.0
        sel_h[RFE_FR + m, N_OUT + r] = 1.0
        sel_h[RFE_ESTEP + o, 2 * N_OUT + r] = 1.0

    rfe = np.concatenate([estep, res, fr], axis=1)  # [T, 16]

    b1v = np.asarray(inputs["b1"], f32).reshape(-1)
    has_b1 = bool(np.any(b1v != 0.0))

    in_maps = []
    for c in range(NCORES):
        sl = slice(c * TC, (c + 1) * TC)
        im = {
            "bbT": np.ascontiguousarray(bb[sl].T.astype(f16)),
            "rfeT": np.ascontiguousarray(rfe[sl].T.astype(f16)),
            "aggT": np.ascontiguousarray(
                np.concatenate([enode[sl], ccl[sl], cnd[sl]], axis=1).T.astype(f16)
            ),
            "w1b": w1b_h,
            "w1a": w1a_h,
            "sel": sel_h,
            "whwa": whwa_h,
        }
        if has_b1:
            im["b1"] = np.ascontiguousarray(b1v.reshape(D_H, 1))
        in_maps.append(im)
    return in_maps


def _assemble(results: list[dict[str, np.ndarray]]) -> np.ndarray:
    parts = [np.asarray(results[c]["out"]).T.reshape(-1) for c in range(NCORES)]
    return np.concatenate(parts)[None, :].astype(np.float32)


def _run(inputs: dict, trace: bool = False):
    b1v = np.asarray(inputs["b1"], np.float32).reshape(-1)
    sc = lambda k: float(np.asarray(inputs[k]).reshape(-1)[0])
    nc = _get_nc(b1v, sc("b3"), sc("b4"), sc("b5"), sc("b6"))
    in_maps = _make_in_maps(inputs)
    kres = run_bass_kernel_spmd(
        nc, in_maps, core_ids=list(range(NCORES)), trace=trace
    )
    return _assemble(kres.results), kres


def kernel(**inputs) -> np.ndarray:
    out, _ = _run(inputs)
    return out
